# revision 74
# baseline (speedup 1.0000x reference)
"""Trainium2 Bass kernel for causal multi-head self-attention.

nn.Module: y = MHSA(x) with D=768, H=12 heads, d_k=64, S=4096, causal mask,
torch-Linear convention (y = x @ W.T, no bias).

Distribution over the 8 NeuronCores (no collectives — host-side gather
between two device launches):

  Launch 1 (same program on all 8 cores): QKV projections, sequence-
  sharded, all-fp16 dataflow (fp32 PSUM accumulation). Core c projects x
  rows [512c, 512c+512) against all of W_q/W_k/W_v, emitting Q^T and K^T
  (head-dim-major) and V (natural). Every tensor moves as one (or two)
  host-packed SBUF-image DMAs so transfers are few and fat. The host
  concatenates the shards.

  Launch 2 (MPMD, one program variant per core): attention + W_o,
  query-sharded with zig-zag causal load balancing: core c owns the two
  256-row query blocks (c, 15-c). Scores are computed transposed
  (scores^T[kv, q], K-tile stationary / Q^T moving, fp16 at full PE
  rate). The strict-upper causal mask is additive (-60000), applied on
  the PE itself via an identity-stationary matmul that accumulates a
  mask tile into the diagonal tiles' PSUM — any DVE/ACT involvement
  here would serialize against neighbouring score matmuls through the
  PSUM bank-overlap tracker. Softmax skips max-subtraction (scores
  ~N(0,1); exp cannot overflow) and gets its denominators for free via
  a ones-column appended to V. The exp is SPLIT across two engines to
  double softmax throughput: each group's scores land in two separate
  PSUM tiles — region A ([128,1024], exact exp on the scalar engine)
  and region D ([128,512], Schraudolph int16-bitcast exp on the DVE,
  one tensor_scalar op) — separate tiles because the bank tracker
  serializes two engines reading one tile. The diagonal-mask tiles are
  stream-ordered to always land in ACT regions (the int16 trick cannot
  represent -60000-masked scores); the diagonal-free last group of each
  head flips its A region to DVE too, balancing ACT/DVE/PE at ~6us per
  head. P^T feeds tiny 65-column AV matmuls producing attention output
  in natural [q, d] layout where the softmax denominators are
  per-partition scalars. Head pairs are transposed back on the PE into
  the W_o contraction layout; W_o finishes and core c returns y^T fp16.

  Scheduling details that matter: DMA->tile dependencies are
  tile-granular (a reader waits for EVERY dma into its tile), so
  anything loaded in stages gets its own tile (per-head Q tiles, two K
  half-tiles); the startup loads only what head 0 needs before the
  first matmul; K/V/Q for head h+1 prefetch during head h; AV of score
  group g is emitted after scores of group g+1 so the in-order PE
  sequencer never head-of-line blocks on exp(g); ~12 dummy identity
  matmuls at build start ramp the PE out of its slow p-states while the
  first DMAs are in flight.

Precision: fp16 everywhere off-chip and for PE operands, fp32 PSUM
accumulation, fp32 softmax reciprocals, ~45% of softmax exps via the
~3%-max-error Schraudolph approximation (the normalization cancels most
of it). End-to-end max error vs the fp32 reference is ~2.6e-3 of the
output absmax (gate: 2e-2).
"""

import numpy as np
import jax

import concourse.tile as tile
import concourse.mybir as mybir
from concourse import bacc, bass2jax

FP16 = mybir.dt.float16
F32 = mybir.dt.float32
I16 = mybir.dt.int16
AF = mybir.ActivationFunctionType
ALU = mybir.AluOpType

# Schraudolph exp in fp16 for the DVE share of the softmax:
#   exp(s*0.125) ~= bitcast_fp16(int16(s*A_SCH + B_SCH))
# (one tensor_scalar op; max rel err ~3.0%, which the softmax
# normalization cancels to ~5e-4 end-to-end).
A_SCH = 0.125 * float(np.log2(np.e)) * 1024.0
B_SCH = 15.0 * 1024.0 - 44.0

B = 1
D = 768          # d_model
S = 4096         # sequence length
H = 12           # heads
DK = 64          # head dim
NC = 8           # NeuronCores
NB = 16          # 256-row query blocks
QB = S // NB     # 256
SC = S // NC     # 512 rows per core
NT = D // 128    # 6
VW = DK + 1      # V augmented with a ones column
NKT = S // 128   # 32 kv tiles

def _blocks_for_core(c):
    return (c, NB - 1 - c)


# --------------------------------------------------------------------------
# MPMD runner: run a (possibly different) bass program on each NeuronCore
# concurrently via the bass_exec custom-call machinery.
# --------------------------------------------------------------------------

def _io_names(nc):
    in_names, out_names, out_avals = [], [], []
    pname = nc.partition_id_tensor.name if nc.partition_id_tensor else None
    for alloc in nc.m.functions[0].allocations:
        if not isinstance(alloc, mybir.MemoryLocationSet):
            continue
        name = alloc.memorylocations[0].name
        if alloc.kind == "ExternalInput":
            if name != pname:
                in_names.append(name)
        elif alloc.kind == "ExternalOutput":
            out_names.append(name)
            out_avals.append(
                jax.core.ShapedArray(
                    tuple(alloc.tensor_shape), mybir.dt.np(alloc.dtype)))
    return in_names, out_names, out_avals, pname


_jit_cache = {}


def run_mpmd(ncs, in_maps):
    """ncs: one compiled Bacc program per core (entries may repeat);
    in_maps: per-core dict name->np.ndarray. Returns per-core output dicts."""
    bass2jax.install_neuronx_cc_hook()
    devices = jax.devices()[: len(ncs)]
    futs, metas = [], []
    for core_id, (nc, in_map, dev) in enumerate(
            zip(ncs, in_maps, devices, strict=True)):
        in_names, out_names, out_avals, pname = _io_names(nc)
        key = (id(nc), core_id)
        if key not in _jit_cache:
            all_names = tuple(in_names + out_names + ([pname] if pname else []))

            def _body(*args, _nc=nc, _avals=tuple(out_avals),
                      _names=all_names, _onames=tuple(out_names)):
                return tuple(bass2jax._bass_exec_p.bind(
                    *args, out_avals=_avals, in_names=_names,
                    out_names=_onames, lowering_input_output_aliases=(),
                    sim_require_finite=True, sim_require_nnan=True, nc=_nc))

            n_params = len(in_names)
            donate = tuple(range(n_params, n_params + len(out_avals)))
            _jit_cache[key] = jax.jit(
                _body, donate_argnums=donate, keep_unused=True)
        fn = _jit_cache[key]
        dev_args = [jax.device_put(np.asarray(in_map[n]), dev)
                    for n in in_names]
        dev_zeros = [jax.device_put(np.zeros(a.shape, a.dtype), dev)
                     for a in out_avals]
        extra = ([jax.device_put(np.array([[core_id]], np.uint32), dev)]
                 if pname else [])
        futs.append(fn(*dev_args, *dev_zeros, *extra))
        metas.append(out_names)
    return [
        {n: np.asarray(a) for n, a in zip(names, arrs, strict=True)}
        for names, arrs in zip(metas, futs)
    ]


# --------------------------------------------------------------------------
# Launch 1: QKV projections (one shared program, SPMD over sequence shards)
# --------------------------------------------------------------------------

def build_qkv():
    """Per-core, all fp16, SBUF-image I/O:
      xI  [128, 6*512]  xI[p, k*512+s]   = x[c*512+s, k*128+p]
      WqP/WkP [128, 6*768] (m,k)-tile-major packed W^T
      WvP [128, 6*768]  WvP[p, k*768+n]  = W_v[n, k*128+p]
      QtI/KtI [128, 6*512] out images (m-tile-major)
      VnI [128, 4*768]  VnI[p, sq*768+n] = V[c*512+sq*128+p, n]
    """
    nc = bacc.Bacc("TRN2", target_bir_lowering=False, debug=False)
    xI = nc.dram_tensor("xI", [128, NT * SC], FP16, kind="ExternalInput").ap()
    WqP = nc.dram_tensor("WqP", [128, NT * D], FP16, kind="ExternalInput").ap()
    WkP = nc.dram_tensor("WkP", [128, NT * D], FP16, kind="ExternalInput").ap()
    WvP = nc.dram_tensor("WvP", [128, NT * D], FP16, kind="ExternalInput").ap()
    QtI = nc.dram_tensor("QtI", [128, NT * SC], FP16, kind="ExternalOutput").ap()
    KtI = nc.dram_tensor("KtI", [128, NT * SC], FP16, kind="ExternalOutput").ap()
    VnI = nc.dram_tensor("VnI", [128, 4 * D], FP16, kind="ExternalOutput").ap()

    with tile.TileContext(nc) as tc:
        with (
            tc.tile_pool(name="xp", bufs=1) as xp,
            tc.tile_pool(name="wp", bufs=3) as wp,
            tc.tile_pool(name="ps", bufs=4, space="PSUM") as ps,
            tc.tile_pool(name="op", bufs=2) as op,
        ):
            # PE p-state warm-up: ~3us of dummy matmuls on the first weight
            # tile while the input DMAs stream in, so every real matmul runs
            # at the full 2.4 GHz clock instead of the 1.2 GHz mid p-state.
            wu_sb = xp.tile([128, 128], FP16, tag="wu")
            nc.sync.dma_start(wu_sb[:], WqP[:, :128])
            with tc.tile_pool(name="ps_w", bufs=1, space="PSUM") as ps_w:
                wu_ps = ps_w.tile([128, 128], F32, tag="wups")
                for _ in range(40):
                    nc.tensor.matmul(wu_ps[:], wu_sb[:], wu_sb[:],
                                     start=True, stop=True)

            # DMA->tile dependencies are tile-granular: a reader waits for
            # EVERY dma into its tile, so anything loaded in stages gets its
            # own tile. Startup order: x k-tiles 0-2, W_q m-tile 0, x 3-5,
            # W_q m-tile 1, W_q rest — the first accumulation chain starts
            # ~5us in and never stalls on a later W load.
            xtf_a = xp.tile([128, 3 * SC], FP16, tag="xtfa")
            xtf_b = xp.tile([128, 3 * SC], FP16, tag="xtfb")
            wq_p0 = wp.tile([128, D], FP16, tag="wq0")
            wq_p1 = wp.tile([128, D], FP16, tag="wq1")
            wq_pr = wp.tile([128, 4 * D], FP16, tag="wqr")
            wq_parts = [wq_p0, wq_p1, wq_pr]
            nc.sync.dma_start(xtf_a[:], xI[:, :3 * SC])
            nc.sync.dma_start(wq_parts[0][:], WqP[:, :D])
            nc.sync.dma_start(wq_parts[1][:], WqP[:, D:2 * D])
            nc.sync.dma_start(xtf_b[:], xI[:, 3 * SC:])
            nc.sync.dma_start(wq_parts[2][:], WqP[:, 2 * D:])

            def xtf(k):
                if k < 3:
                    return xtf_a[:, k * SC:(k + 1) * SC]
                return xtf_b[:, (k - 3) * SC:(k - 2) * SC]

            def wq_tile(m, k):
                if m < 2:
                    return wq_parts[m][:, k * 128:(k + 1) * 128]
                return wq_parts[2][:, (m - 2) * D + k * 128:
                                   (m - 2) * D + (k + 1) * 128]

            # Q^T / K^T: out tile m = sum_k Wp[m][k]^T @ x^T[k]
            # (per-m output tiles + DMAs keep the section tails small)
            for W_ap, out_ap, nm in ((WqP, QtI, "q"), (WkP, KtI, "k")):
                if nm == "q":
                    w_tile = wq_tile
                else:
                    w_sb = wp.tile([128, NT * D], FP16, tag="w" + nm)
                    nc.sync.dma_start(w_sb[:], W_ap[:])

                    def w_tile(m, k, _w=w_sb):
                        return _w[:, m * D + k * 128:m * D + (k + 1) * 128]
                oI = op.tile([128, NT * SC], FP16, tag="o" + nm,
                             name="oI" + nm)
                for m in range(NT):
                    acc = ps.tile([128, SC], F32, tag="acc")
                    for k in range(NT):
                        nc.tensor.matmul(
                            acc[:], w_tile(m, k),
                            xtf(k), start=(k == 0), stop=(k == NT - 1))
                    nc.vector.tensor_copy(oI[:, m * SC:(m + 1) * SC], acc[:])
                nc.sync.dma_start(out_ap[:], oI[:])
            wv_sb = wp.tile([128, NT * D], FP16, tag="wv")
            nc.sync.dma_start(wv_sb[:], WvP[:])
            for sq in range(4):
                split = sq == 3
                vI = None if split else op.tile([128, D], FP16, tag="vI")
                for n0, n1 in ((0, 384), (384, 768)):
                    acc = ps.tile([128, n1 - n0], F32, tag="acc")
                    for k in range(NT):
                        nc.tensor.matmul(
                            acc[:],
                            xtf(k)[:, sq * 128:(sq + 1) * 128],
                            wv_sb[:, k * D + n0:k * D + n1],
                            start=(k == 0), stop=(k == NT - 1))
                    if split:
                        vH = op.tile([128, n1 - n0], FP16, tag="vH",
                                     name=f"vH{n0}")
                        nc.vector.tensor_copy(vH[:], acc[:])
                        nc.sync.dma_start(
                            VnI[:, sq * D + n0:sq * D + n1], vH[:])
                    else:
                        nc.vector.tensor_copy(vI[:, n0:n1], acc[:])
                if not split:
                    nc.sync.dma_start(VnI[:, sq * D:(sq + 1) * D], vI[:])
    nc.compile()
    return nc


# --------------------------------------------------------------------------
# Launch 2: attention + W_o (one program variant per core)
# --------------------------------------------------------------------------

def build_attn(core):
    bA, bB = _blocks_for_core(core)
    tA, tB = 2 * bA + 2, 2 * bB + 2   # causal kv-tile counts per block
    SG = 3   # shared-range kv tiles per exp group ([128,1536] = 3 banks)

    nc = bacc.Bacc("TRN2", target_bir_lowering=False, debug=False)
    Qt = nc.dram_tensor("Qt", [DK, H * SC], FP16, kind="ExternalInput").ap()
    Kt = nc.dram_tensor("Kt", [D, S], FP16, kind="ExternalInput").ap()
    # per-head SBUF image of augmented V: Vh[p, h*2080 + t*65 + e]
    Vh = nc.dram_tensor("Vh", [128, H * NKT * VW], FP16,
                        kind="ExternalInput").ap()
    WoT = nc.dram_tensor("WoT", [D, D], FP16, kind="ExternalInput").ap()
    # statics image: [0:256) = mb0, [256:512) = mb1 (0/1 fp16 causal masks),
    # [512:640) = 128x128 identity
    St = nc.dram_tensor("St", [128, 2 * QB + 128], FP16,
                        kind="ExternalInput").ap()
    yT = nc.dram_tensor("yT", [D, SC], FP16, kind="ExternalOutput").ap()

    # packed score stream per head: shared-range tiles (both blocks, 512
    # wide) first, then B-only tiles (256 wide), bin-packed into
    # [128, 1536] groups so exp runs in 6 ACTIVATEs/head. The four causal
    # diagonal tiles are pulled to the front of the stream (right after
    # the non-diagonal shared tiles) so they always land in the exact-exp
    # (ACT) prefix of their group, never in the Schraudolph suffix, whose
    # int16 trick cannot represent the -60000-masked scores.
    # Stream order puts the A-block diagonal pair first (it fills group 0's
    # A region exactly), then just enough non-diagonal tiles to fill group
    # 0's D region, then the B-block diagonal pair (start of group 1's A
    # region), then everything else. Diagonals land in ACT-exp'd A regions
    # and the last group is diagonal-free on every core.
    nondiag = ([t for t in range(tA - 2)]
               + [t for t in range(tA, tB - 2)])
    j, wsum = 0, 0
    while wsum < SC:
        wsum += SC if nondiag[j] < tA else QB
        j += 1
    order = ([tA - 2, tA - 1] + nondiag[:j] + [tB - 2, tB - 1]
             + nondiag[j:])
    raw_groups, cur, off = [], [], 0
    for t in order:
        w = SC if t < tA else QB
        diag = t in (tA - 2, tA - 1, tB - 2, tB - 1)
        if off + w > SG * SC:
            raw_groups.append(cur)
            cur, off = [], 0
        cur.append((t, w, diag))
        off += w
    if cur:
        raw_groups.append(cur)
    t_first, t_last = order[0], order[-1]

    # Each group's tiles are split across TWO separate PSUM score tiles:
    # region A ([128,1024], exact exp on ACT) and region D ([128,512],
    # Schraudolph exp on DVE — unless a masked diagonal tile lands there,
    # in which case ACT handles D too). Separate tiles/banks per engine:
    # a single shared tile would serialize the ACT and DVE reads through
    # the PSUM bank-overlap tracker. Entries: (t, region, roff, w, diag).
    groups = []
    for cur in raw_groups:
        total = sum(w for _, w, _ in cur)
        dwidth, nd = 0, 0
        for t, w, diag in reversed(cur):
            if dwidth + w > SC:
                break
            dwidth += w
            nd += 1
        asz = total - dwidth
        assert asz <= 2 * SC and dwidth <= SC
        grp, aoff, doff = [], 0, 0
        for i, (t, w, diag) in enumerate(cur):
            if i < len(cur) - nd:
                grp.append((t, "A", aoff, w, diag))
                aoff += w
            else:
                grp.append((t, "D", doff, w, diag))
                doff += w
        groups.append((grp, aoff, doff,
                       any(dg for _, r, _, _, dg in grp if r == "D"),
                       any(dg for _, _, _, _, dg in grp)))

    with tile.TileContext(nc) as tc:
        with (
            tc.tile_pool(name="stat", bufs=1) as stat,
            tc.tile_pool(name="kp", bufs=2) as kp,
            tc.tile_pool(name="qp", bufs=3) as qp,
            tc.tile_pool(name="vp", bufs=2) as vp,
            tc.tile_pool(name="pp", bufs=8) as pp,
            tc.tile_pool(name="ppd", bufs=8) as ppd,
            tc.tile_pool(name="dp", bufs=4) as dp,
        ):
            # startup-critical loads first: statics, Q/K/V for head 0 only.
            st_sb = stat.tile([128, 2 * QB + 128], FP16, tag="st")
            nc.sync.dma_start(st_sb[:], St[:])
            mb0 = st_sb[:, 0:QB]
            mb1 = st_sb[:, QB:2 * QB]
            id_sb = st_sb[:, 2 * QB:]

            # DMA->tile deps are tile-granular (a reader waits for every DMA
            # into its tile), so Q gets one tile per head and K two per head.
            q_tiles, kv_tiles = {}, {}

            def load_q(h):
                qt_h = qp.tile([64, SC], FP16, tag="qt")
                nc.sync.dma_start(qt_h[:], Qt[:, h * SC:(h + 1) * SC])
                q_tiles[h] = qt_h

            load_q(0)

            # PE p-state warm-up during the K/V head-0 loads (see build_qkv)
            with tc.tile_pool(name="ps_w", bufs=1, space="PSUM") as ps_w:
                wu_ps = ps_w.tile([128, 128], F32, tag="wups")
                for _ in range(12):
                    nc.tensor.matmul(wu_ps[:], id_sb, id_sb,
                                     start=True, stop=True)

            def load_kv(h):
                kt_a = kp.tile([64, S // 2], FP16, tag="kta")
                nc.sync.dma_start(kt_a[:], Kt[h * 64:(h + 1) * 64, :S // 2])
                kt_b = kp.tile([64, S // 2], FP16, tag="ktb")
                nc.sync.dma_start(kt_b[:], Kt[h * 64:(h + 1) * 64, S // 2:])
                v_h = vp.tile([128, NKT * VW], FP16, tag="v")
                nc.sync.dma_start(
                    v_h[:], Vh[:, h * NKT * VW:(h + 1) * NKT * VW])
                kv_tiles[h] = (kt_a, kt_b, v_h)

            load_kv(0)

            # normalized attention output, natural layout:
            # [128 q, (qsub, h*64+d)] fp16
            attn_nat = stat.tile([128, 4 * D], FP16, tag="attn_nat")
            attn_bf = stat.tile([128, NT * SC], FP16, tag="attn")
            wot_sb = stat.tile([128, NT * D], FP16, tag="wot")

            def q_rhs(qt_h, qo, width):
                return qt_h[:, qo:qo + width]

            with (
                tc.tile_pool(name="ps_sa", bufs=2, space="PSUM") as ps_sa,
                tc.tile_pool(name="ps_sd", bufs=2, space="PSUM") as ps_sd,
                tc.tile_pool(name="ps_u", bufs=1, space="PSUM") as ps_u,
                tc.tile_pool(name="ps_t", bufs=1, space="PSUM") as ps_t,
            ):
                pending_fin = []
                for h in range(H):
                    kt_a, kt_b, v_h = kv_tiles.pop(h)
                    qt_h = q_tiles.pop(h)
                    # natural-layout AV accumulators, one per 128-q
                    # sub-tile, all four in ONE psum bank (4*65 = 260 f32).
                    # Only the very first mm uses start=True: it marks the
                    # whole 2KB bank pending-zero; the first write to each
                    # byte then overwrites, later writes accumulate.
                    unat = ps_u.tile([128, 512], F32, tag="u")

                    def av(t, p_slice, block, sub, _u=unat, _v=v_h):
                        uqo = (block * 2 + sub) * VW
                        nc.tensor.matmul(
                            _u[:, uqo:uqo + VW],
                            p_slice,
                            _v[:, t * VW:(t + 1) * VW],
                            start=(t == t_first and sub == 0 and block == 0),
                            stop=(t == t_last and block == 1 and sub == 1),
                            skip_group_check=True)

                    def emit_av(grp, p_act, p_dve):
                        for t, reg, roff, w, _ in grp:
                            p = p_act if reg == "A" else p_dve
                            for sub in (0, 1):
                                if w == SC:
                                    av(t, p[:, roff + sub * 128:
                                           roff + (sub + 1) * 128], 0, sub)
                                    av(t, p[:, roff + QB + sub * 128:
                                           roff + QB + (sub + 1) * 128],
                                       1, sub)
                                else:
                                    av(t, p[:, roff + sub * 128:
                                           roff + (sub + 1) * 128], 1, sub)

                    pends = []   # AV of group g is deferred four groups
                    for gi, (grp, wA, wD, d_diag, any_diag) in \
                            enumerate(groups):
                        sc_a = ps_sa.tile([128, 2 * SC], F32, tag="sa")
                        sc_d = ps_sd.tile([128, SC], F32, tag="sd")
                        for t, reg, roff, w, dg in grp:
                            sc = sc_a if reg == "A" else sc_d
                            nc.tensor.matmul(
                                sc[:, roff:roff + w],
                                (kt_a if t < NKT // 2 else kt_b)[
                                    :, (t % (NKT // 2)) * 128:
                                    (t % (NKT // 2) + 1) * 128],
                                q_rhs(qt_h, 0 if w == SC else QB, w),
                                start=True, stop=not dg)
                            if dg:
                                # additive causal mask (-60000 on the strict
                                # upper triangle), applied on the PE itself:
                                # identity-stationary matmul accumulates the
                                # mask tile into this score region
                                nc.tensor.matmul(
                                    sc[:, roff:roff + QB],
                                    id_sb,
                                    mb0 if t in (tA - 2, tB - 2) else mb1,
                                    start=False, stop=True,
                                    skip_group_check=True)
                        p_act = pp.tile([128, 2 * SC], FP16, tag="p")
                        p_dve = ppd.tile([128, SC], FP16, tag="pd")
                        # the (always diagonal-free) last group's A region
                        # also goes to DVE: evens out the ACT/DVE load
                        if gi == len(groups) - 1 and not any_diag:
                            nc.vector.tensor_scalar(
                                p_act[:, :wA].bitcast(I16),
                                sc_a[:, :wA], A_SCH, B_SCH,
                                op0=ALU.mult, op1=ALU.add)
                        else:
                            nc.scalar.activation(
                                p_act[:, :wA], sc_a[:, :wA], AF.Exp,
                                scale=0.125)
                        if wD:
                            if d_diag:
                                nc.scalar.activation(
                                    p_dve[:, :wD], sc_d[:, :wD], AF.Exp,
                                    scale=0.125)
                            else:
                                nc.vector.tensor_scalar(
                                    p_dve[:, :wD].bitcast(I16),
                                    sc_d[:, :wD], A_SCH, B_SCH,
                                    op0=ALU.mult, op1=ALU.add)
                        # prefetches + the previous head's deferred norm
                        # ride behind head h's first score group (emitting
                        # norm(h-1) here keeps it from head-of-line blocking
                        # head h's DVE exps in the in-order DVE queue)
                        if gi == 0:
                            if h + 1 < H:
                                load_q(h + 1)
                                load_kv(h + 1)
                            if h == 0:
                                for g in range(NT):
                                    nc.sync.dma_start(
                                        wot_sb[:, g * D:(g + 1) * D],
                                        WoT[g * 128:(g + 1) * 128, :])
                            if pending_fin:
                                pending_fin.pop()()

                        pends.append((grp, p_act, p_dve))
                        if len(pends) > 4:
                            emit_av(*pends.pop(0))
                    for pd in pends:
                        emit_av(*pd)

                    def finish_head(h=h, unat=unat):
                        # normalize: denominators are per-partition scalars
                        for block, sub in ((0, 0), (0, 1), (1, 0), (1, 1)):
                            qsub = block * 2 + sub
                            uqo = qsub * VW
                            r = dp.tile([128, 1], F32, tag="recip")
                            nc.vector.reciprocal(
                                r[:], unat[:, uqo + 64:uqo + 65])
                            nc.vector.tensor_scalar_mul(
                                attn_nat[:, qsub * D + h * DK:
                                         qsub * D + (h + 1) * DK],
                                unat[:, uqo:uqo + 64], r[:])
                        # transpose the finished head pair into W_o layout
                        if h % 2 == 1:
                            g = h // 2
                            for qsub in range(4):
                                tps = ps_t.tile([128, 128], FP16, tag="t")
                                nc.tensor.transpose(
                                    tps[:],
                                    attn_nat[:, qsub * D + g * 128:
                                             qsub * D + (g + 1) * 128],
                                    id_sb)
                                nc.vector.tensor_copy(
                                    attn_bf[:, g * SC + qsub * 128:
                                            g * SC + (qsub + 1) * 128],
                                    tps[:])

                    pending_fin.append(finish_head)
                if pending_fin:
                    pending_fin.pop()()

            # W_o: y^T[o-tile] = sum_c WoT[c-tile, o-tile]^T @ attn^T[c-tile].
            # ct-outer with all six o-accumulators live: the 30 matmuls over
            # head pairs 0-4 never sit in-order behind a chain that needs the
            # just-finished pair 5, so only the last 6 matmuls are exposed.
            with (
                tc.tile_pool(name="ps_y", bufs=4, space="PSUM") as ps_y,
                tc.tile_pool(name="yo", bufs=3) as yo,
            ):
                for o in range(NT):
                    yps = ps_y.tile([128, SC], F32, tag="y")
                    for ct in range(NT):
                        nc.tensor.matmul(
                            yps[:],
                            wot_sb[:, ct * D + o * 128:ct * D + (o + 1) * 128],
                            attn_bf[:, ct * SC:(ct + 1) * SC],
                            start=(ct == 0), stop=(ct == NT - 1))
                    yt_sb = yo.tile([128, SC], FP16, tag="yt")
                    nc.vector.tensor_copy(yt_sb[:], yps[:])
                    nc.sync.dma_start(yT[o * 128:(o + 1) * 128, :], yt_sb[:])
    nc.compile()
    return nc


# --------------------------------------------------------------------------
# Host-side packing + the public entry point
# --------------------------------------------------------------------------

def _make_statics():
    r = np.arange(128)[:, None]
    j = np.arange(QB)[None, :]
    st = np.empty((128, 2 * QB + 128), np.float16)
    st[:, 0:QB] = np.where(r > j, -60000.0, 0.0)          # additive mask m0
    st[:, QB:2 * QB] = np.where(128 + r > j, -60000.0, 0.0)  # m1
    st[:, 2 * QB:] = np.eye(128)
    return st


_programs = None


def _get_programs():
    global _programs
    if _programs is None:
        qkv = build_qkv()
        attn = [build_attn(c) for c in range(NC)]
        _programs = (qkv, attn)
    return _programs


def _pack_w_mk(W):
    """[768,768] torch Linear weight -> [128, 36*128] fp16, (m,k)-tile-major:
    out[p, (m*6+k)*128 + j] = W[m*128+j, k*128+p] (i.e. W^T by tiles)."""
    Wt = np.asarray(W, np.float32).T.astype(np.float16)       # [in k, out m]
    t = Wt.reshape(NT, 128, NT, 128)                           # [k, p, m, j]
    return np.ascontiguousarray(
        t.transpose(1, 2, 0, 3).reshape(128, NT * D))          # [p, m, k, j]


def kernel(x, W_q, W_k, W_v, W_o):
    x = np.asarray(x)
    in_dtype = x.dtype
    xs = np.asarray(x, np.float32).reshape(S, D)
    qkv_nc, attn_ncs = _get_programs()

    # ---- launch 1: QKV projections, sequence-sharded ----
    WqP, WkP = _pack_w_mk(W_q), _pack_w_mk(W_k)
    # WvP[p, k*768+n] = W_v[n, k*128+p]
    WvP = np.ascontiguousarray(
        np.asarray(W_v, np.float32).T.astype(np.float16)
        .reshape(NT, 128, D).transpose(1, 0, 2).reshape(128, NT * D))
    x16 = xs.astype(np.float16)
    in_maps1 = []
    for c in range(NC):
        xc = x16[c * SC:(c + 1) * SC]                          # [512, 768]
        xIc = np.ascontiguousarray(
            xc.T.reshape(NT, 128, SC).transpose(1, 0, 2).reshape(128, NT * SC))
        in_maps1.append({"xI": xIc, "WqP": WqP, "WkP": WkP, "WvP": WvP})
    res1 = run_mpmd([qkv_nc] * NC, in_maps1)

    # ---- host gather + repack ----
    # QtI/KtI images -> [768, 4096]; VnI image -> [4096, 768]
    Qt_full = np.empty((D, S), np.float16)
    Kt_full = np.empty((D, S), np.float16)
    V_full = np.empty((S, D), np.float16)
    for c in range(NC):
        q = res1[c]["QtI"].reshape(128, NT, SC).transpose(1, 0, 2)
        k = res1[c]["KtI"].reshape(128, NT, SC).transpose(1, 0, 2)
        Qt_full[:, c * SC:(c + 1) * SC] = q.reshape(D, SC)
        Kt_full[:, c * SC:(c + 1) * SC] = k.reshape(D, SC)
        v = res1[c]["VnI"].reshape(128, 4, D).transpose(1, 0, 2)
        V_full[c * SC:(c + 1) * SC] = v.reshape(SC, D)
    # per-head SBUF image of V augmented with a ones column:
    # Vh[p, h, t, e] = Vaug[t*128+p, h*65+e]
    Vaug = np.empty((NKT, 128, H, VW), np.float16)
    Vaug[:, :, :, :64] = V_full.reshape(NKT, 128, H, 64)
    Vaug[:, :, :, 64] = np.float16(1.0)
    Vh = np.ascontiguousarray(
        Vaug.transpose(1, 2, 0, 3).reshape(128, H * NKT * VW))
    st = _make_statics()

    # ---- launch 2: attention + W_o, query-sharded (zig-zag) ----
    WoT = np.ascontiguousarray(np.asarray(W_o, np.float32).T).astype(np.float16)
    in_maps2 = []
    for c in range(NC):
        bA, bB = _blocks_for_core(c)
        # per-head [64, 512] with that core's two query blocks side by side
        qh = np.empty((DK, H * SC), np.float16)
        for h in range(H):
            qh[:, h * SC:h * SC + QB] = \
                Qt_full[h * DK:(h + 1) * DK, bA * QB:(bA + 1) * QB]
            qh[:, h * SC + QB:(h + 1) * SC] = \
                Qt_full[h * DK:(h + 1) * DK, bB * QB:(bB + 1) * QB]
        in_maps2.append({
            "Qt": qh, "Kt": Kt_full, "Vh": Vh, "WoT": WoT, "St": st,
        })
    res2 = run_mpmd(attn_ncs, in_maps2)

    # ---- host scatter ----
    y = np.empty((S, D), np.float32)
    for c in range(NC):
        bA, bB = _blocks_for_core(c)
        yc = res2[c]["yT"].T.astype(np.float32)  # [512, 768]
        y[bA * QB:(bA + 1) * QB] = yc[:QB]
        y[bB * QB:(bB + 1) * QB] = yc[QB:]
    return y.reshape(B, S, D).astype(in_dtype, copy=False)


# revision 75
# speedup vs baseline: 1.0015x; 1.0015x over previous
"""Trainium2 Bass kernel for causal multi-head self-attention.

nn.Module: y = MHSA(x) with D=768, H=12 heads, d_k=64, S=4096, causal mask,
torch-Linear convention (y = x @ W.T, no bias).

Distribution over the 8 NeuronCores (no collectives — host-side gather
between two device launches):

  Launch 1 (same program on all 8 cores): QKV projections, sequence-
  sharded, all-fp16 dataflow (fp32 PSUM accumulation). Core c projects x
  rows [512c, 512c+512) against all of W_q/W_k/W_v, emitting Q^T and K^T
  (head-dim-major) and V (natural). Every tensor moves as one (or two)
  host-packed SBUF-image DMAs so transfers are few and fat. The host
  concatenates the shards.

  Launch 2 (MPMD, one program variant per core): attention + W_o,
  query-sharded with zig-zag causal load balancing: core c owns the two
  256-row query blocks (c, 15-c). Scores are computed transposed
  (scores^T[kv, q], K-tile stationary / Q^T moving, fp16 at full PE
  rate). The strict-upper causal mask is additive (-60000), applied on
  the PE itself via an identity-stationary matmul that accumulates a
  mask tile into the diagonal tiles' PSUM — any DVE/ACT involvement
  here would serialize against neighbouring score matmuls through the
  PSUM bank-overlap tracker. Softmax skips max-subtraction (scores
  ~N(0,1); exp cannot overflow) and gets its denominators for free via
  a ones-column appended to V. The exp is SPLIT across two engines to
  double softmax throughput: each group's scores land in two separate
  PSUM tiles — region A ([128,1024], exact exp on the scalar engine)
  and region D ([128,512], Schraudolph int16-bitcast exp on the DVE,
  one tensor_scalar op) — separate tiles because the bank tracker
  serializes two engines reading one tile. The diagonal-mask tiles are
  stream-ordered to always land in ACT regions (the int16 trick cannot
  represent -60000-masked scores); the diagonal-free last group of each
  head flips its A region to DVE too, balancing ACT/DVE/PE at ~6us per
  head. P^T feeds tiny 65-column AV matmuls producing attention output
  in natural [q, d] layout where the softmax denominators are
  per-partition scalars. Head pairs are transposed back on the PE into
  the W_o contraction layout; W_o finishes and core c returns y^T fp16.

  Scheduling details that matter: DMA->tile dependencies are
  tile-granular (a reader waits for EVERY dma into its tile), so
  anything loaded in stages gets its own tile (per-head Q tiles, two K
  half-tiles); the startup loads only what head 0 needs before the
  first matmul; K/V/Q for head h+1 prefetch during head h; AV of score
  group g is emitted after scores of group g+1 so the in-order PE
  sequencer never head-of-line blocks on exp(g); ~12 dummy identity
  matmuls at build start ramp the PE out of its slow p-states while the
  first DMAs are in flight.

Precision: fp16 everywhere off-chip and for PE operands, fp32 PSUM
accumulation, fp32 softmax reciprocals, ~45% of softmax exps via the
~3%-max-error Schraudolph approximation (the normalization cancels most
of it). End-to-end max error vs the fp32 reference is ~2.6e-3 of the
output absmax (gate: 2e-2).
"""

import numpy as np
import jax

import concourse.tile as tile
import concourse.mybir as mybir
from concourse import bacc, bass2jax

FP16 = mybir.dt.float16
F32 = mybir.dt.float32
I16 = mybir.dt.int16
AF = mybir.ActivationFunctionType
ALU = mybir.AluOpType

# Schraudolph exp in fp16 for the DVE share of the softmax:
#   exp(s*0.125) ~= bitcast_fp16(int16(s*A_SCH + B_SCH))
# (one tensor_scalar op; max rel err ~3.0%, which the softmax
# normalization cancels to ~5e-4 end-to-end).
A_SCH = 0.125 * float(np.log2(np.e)) * 1024.0
B_SCH = 15.0 * 1024.0 - 44.0

B = 1
D = 768          # d_model
S = 4096         # sequence length
H = 12           # heads
DK = 64          # head dim
NC = 8           # NeuronCores
NB = 16          # 256-row query blocks
QB = S // NB     # 256
SC = S // NC     # 512 rows per core
NT = D // 128    # 6
VW = DK + 1      # V augmented with a ones column
NKT = S // 128   # 32 kv tiles

def _blocks_for_core(c):
    return (c, NB - 1 - c)


# --------------------------------------------------------------------------
# MPMD runner: run a (possibly different) bass program on each NeuronCore
# concurrently via the bass_exec custom-call machinery.
# --------------------------------------------------------------------------

def _io_names(nc):
    in_names, out_names, out_avals = [], [], []
    pname = nc.partition_id_tensor.name if nc.partition_id_tensor else None
    for alloc in nc.m.functions[0].allocations:
        if not isinstance(alloc, mybir.MemoryLocationSet):
            continue
        name = alloc.memorylocations[0].name
        if alloc.kind == "ExternalInput":
            if name != pname:
                in_names.append(name)
        elif alloc.kind == "ExternalOutput":
            out_names.append(name)
            out_avals.append(
                jax.core.ShapedArray(
                    tuple(alloc.tensor_shape), mybir.dt.np(alloc.dtype)))
    return in_names, out_names, out_avals, pname


_jit_cache = {}


def run_mpmd(ncs, in_maps):
    """ncs: one compiled Bacc program per core (entries may repeat);
    in_maps: per-core dict name->np.ndarray. Returns per-core output dicts."""
    bass2jax.install_neuronx_cc_hook()
    devices = jax.devices()[: len(ncs)]
    futs, metas = [], []
    for core_id, (nc, in_map, dev) in enumerate(
            zip(ncs, in_maps, devices, strict=True)):
        in_names, out_names, out_avals, pname = _io_names(nc)
        key = (id(nc), core_id)
        if key not in _jit_cache:
            all_names = tuple(in_names + out_names + ([pname] if pname else []))

            def _body(*args, _nc=nc, _avals=tuple(out_avals),
                      _names=all_names, _onames=tuple(out_names)):
                return tuple(bass2jax._bass_exec_p.bind(
                    *args, out_avals=_avals, in_names=_names,
                    out_names=_onames, lowering_input_output_aliases=(),
                    sim_require_finite=True, sim_require_nnan=True, nc=_nc))

            n_params = len(in_names)
            donate = tuple(range(n_params, n_params + len(out_avals)))
            _jit_cache[key] = jax.jit(
                _body, donate_argnums=donate, keep_unused=True)
        fn = _jit_cache[key]
        dev_args = [jax.device_put(np.asarray(in_map[n]), dev)
                    for n in in_names]
        dev_zeros = [jax.device_put(np.zeros(a.shape, a.dtype), dev)
                     for a in out_avals]
        extra = ([jax.device_put(np.array([[core_id]], np.uint32), dev)]
                 if pname else [])
        futs.append(fn(*dev_args, *dev_zeros, *extra))
        metas.append(out_names)
    return [
        {n: np.asarray(a) for n, a in zip(names, arrs, strict=True)}
        for names, arrs in zip(metas, futs)
    ]


# --------------------------------------------------------------------------
# Launch 1: QKV projections (one shared program, SPMD over sequence shards)
# --------------------------------------------------------------------------

def build_qkv():
    """Per-core, all fp16, SBUF-image I/O:
      xI  [128, 6*512]  xI[p, k*512+s]   = x[c*512+s, k*128+p]
      WqP/WkP [128, 6*768] (m,k)-tile-major packed W^T
      WvP [128, 6*768]  WvP[p, k*768+n]  = W_v[n, k*128+p]
      QtI/KtI [128, 6*512] out images (m-tile-major)
      VnI [128, 4*768]  VnI[p, sq*768+n] = V[c*512+sq*128+p, n]
    """
    nc = bacc.Bacc("TRN2", target_bir_lowering=False, debug=False)
    xI = nc.dram_tensor("xI", [128, NT * SC], FP16, kind="ExternalInput").ap()
    WqP = nc.dram_tensor("WqP", [128, NT * D], FP16, kind="ExternalInput").ap()
    WkP = nc.dram_tensor("WkP", [128, NT * D], FP16, kind="ExternalInput").ap()
    WvP = nc.dram_tensor("WvP", [128, NT * D], FP16, kind="ExternalInput").ap()
    QtI = nc.dram_tensor("QtI", [128, NT * SC], FP16, kind="ExternalOutput").ap()
    KtI = nc.dram_tensor("KtI", [128, NT * SC], FP16, kind="ExternalOutput").ap()
    VnI = nc.dram_tensor("VnI", [128, 4 * D], FP16, kind="ExternalOutput").ap()

    with tile.TileContext(nc) as tc:
        with (
            tc.tile_pool(name="xp", bufs=1) as xp,
            tc.tile_pool(name="wp", bufs=3) as wp,
            tc.tile_pool(name="ps", bufs=4, space="PSUM") as ps,
            tc.tile_pool(name="op", bufs=2) as op,
        ):
            # PE p-state warm-up: ~3us of dummy matmuls on the first weight
            # tile while the input DMAs stream in, so every real matmul runs
            # at the full 2.4 GHz clock instead of the 1.2 GHz mid p-state.
            wu_sb = xp.tile([128, 128], FP16, tag="wu")
            nc.sync.dma_start(wu_sb[:], WqP[:, :128])
            with tc.tile_pool(name="ps_w", bufs=1, space="PSUM") as ps_w:
                wu_ps = ps_w.tile([128, 128], F32, tag="wups")
                for _ in range(40):
                    nc.tensor.matmul(wu_ps[:], wu_sb[:], wu_sb[:],
                                     start=True, stop=True)

            # DMA->tile dependencies are tile-granular: a reader waits for
            # EVERY dma into its tile, so anything loaded in stages gets its
            # own tile. Startup order: x k-tiles 0-2, W_q m-tile 0, x 3-5,
            # W_q m-tile 1, W_q rest — the first accumulation chain starts
            # ~5us in and never stalls on a later W load.
            xtf_a = xp.tile([128, 3 * SC], FP16, tag="xtfa")
            xtf_b = xp.tile([128, 3 * SC], FP16, tag="xtfb")
            wq_p0 = wp.tile([128, D], FP16, tag="wq0")
            wq_p1 = wp.tile([128, D], FP16, tag="wq1")
            wq_pr = wp.tile([128, 4 * D], FP16, tag="wqr")
            wq_parts = [wq_p0, wq_p1, wq_pr]
            nc.sync.dma_start(xtf_a[:], xI[:, :3 * SC])
            nc.sync.dma_start(wq_parts[0][:], WqP[:, :D])
            nc.sync.dma_start(wq_parts[1][:], WqP[:, D:2 * D])
            nc.sync.dma_start(xtf_b[:], xI[:, 3 * SC:])
            nc.sync.dma_start(wq_parts[2][:], WqP[:, 2 * D:])

            def xtf(k):
                if k < 3:
                    return xtf_a[:, k * SC:(k + 1) * SC]
                return xtf_b[:, (k - 3) * SC:(k - 2) * SC]

            def wq_tile(m, k):
                if m < 2:
                    return wq_parts[m][:, k * 128:(k + 1) * 128]
                return wq_parts[2][:, (m - 2) * D + k * 128:
                                   (m - 2) * D + (k + 1) * 128]

            # Q^T / K^T: out tile m = sum_k Wp[m][k]^T @ x^T[k]
            # (per-m output tiles + DMAs keep the section tails small)
            for W_ap, out_ap, nm in ((WqP, QtI, "q"), (WkP, KtI, "k")):
                if nm == "q":
                    w_tile = wq_tile
                else:
                    w_sb = wp.tile([128, NT * D], FP16, tag="w" + nm)
                    nc.sync.dma_start(w_sb[:], W_ap[:])

                    def w_tile(m, k, _w=w_sb):
                        return _w[:, m * D + k * 128:m * D + (k + 1) * 128]
                oI = op.tile([128, NT * SC], FP16, tag="o" + nm,
                             name="oI" + nm)
                for m in range(NT):
                    acc = ps.tile([128, SC], F32, tag="acc")
                    for k in range(NT):
                        nc.tensor.matmul(
                            acc[:], w_tile(m, k),
                            xtf(k), start=(k == 0), stop=(k == NT - 1))
                    nc.vector.tensor_copy(oI[:, m * SC:(m + 1) * SC], acc[:])
                nc.sync.dma_start(out_ap[:], oI[:])
            wv_sb = wp.tile([128, NT * D], FP16, tag="wv")
            nc.sync.dma_start(wv_sb[:], WvP[:])
            for sq in range(4):
                split = sq == 3
                vI = None if split else op.tile([128, D], FP16, tag="vI")
                for n0, n1 in ((0, 384), (384, 768)):
                    acc = ps.tile([128, n1 - n0], F32, tag="acc")
                    for k in range(NT):
                        nc.tensor.matmul(
                            acc[:],
                            xtf(k)[:, sq * 128:(sq + 1) * 128],
                            wv_sb[:, k * D + n0:k * D + n1],
                            start=(k == 0), stop=(k == NT - 1))
                    if split:
                        vH = op.tile([128, n1 - n0], FP16, tag="vH",
                                     name=f"vH{n0}")
                        nc.vector.tensor_copy(vH[:], acc[:])
                        nc.sync.dma_start(
                            VnI[:, sq * D + n0:sq * D + n1], vH[:])
                    else:
                        nc.vector.tensor_copy(vI[:, n0:n1], acc[:])
                if not split:
                    nc.sync.dma_start(VnI[:, sq * D:(sq + 1) * D], vI[:])
    nc.compile()
    return nc


# --------------------------------------------------------------------------
# Launch 2: attention + W_o (one program variant per core)
# --------------------------------------------------------------------------

def build_attn(core):
    bA, bB = _blocks_for_core(core)
    tA, tB = 2 * bA + 2, 2 * bB + 2   # causal kv-tile counts per block
    SG = 3   # shared-range kv tiles per exp group ([128,1536] = 3 banks)

    nc = bacc.Bacc("TRN2", target_bir_lowering=False, debug=False)
    Qt = nc.dram_tensor("Qt", [DK, H * SC], FP16, kind="ExternalInput").ap()
    Kt = nc.dram_tensor("Kt", [D, S], FP16, kind="ExternalInput").ap()
    # per-head SBUF image of augmented V: Vh[p, h*2080 + t*65 + e]
    Vh = nc.dram_tensor("Vh", [128, H * NKT * VW], FP16,
                        kind="ExternalInput").ap()
    WoT = nc.dram_tensor("WoT", [D, D], FP16, kind="ExternalInput").ap()
    # statics image: [0:256) = mb0, [256:512) = mb1 (0/1 fp16 causal masks),
    # [512:640) = 128x128 identity
    St = nc.dram_tensor("St", [128, 2 * QB + 128], FP16,
                        kind="ExternalInput").ap()
    yT = nc.dram_tensor("yT", [D, SC], FP16, kind="ExternalOutput").ap()

    # packed score stream per head: shared-range tiles (both blocks, 512
    # wide) first, then B-only tiles (256 wide), bin-packed into
    # [128, 1536] groups so exp runs in 6 ACTIVATEs/head. The four causal
    # diagonal tiles are pulled to the front of the stream (right after
    # the non-diagonal shared tiles) so they always land in the exact-exp
    # (ACT) prefix of their group, never in the Schraudolph suffix, whose
    # int16 trick cannot represent the -60000-masked scores.
    # Stream order puts the A-block diagonal pair first (it fills group 0's
    # A region exactly), then just enough non-diagonal tiles to fill group
    # 0's D region, then the B-block diagonal pair (start of group 1's A
    # region), then everything else. Diagonals land in ACT-exp'd A regions
    # and the last group is diagonal-free on every core.
    nondiag = ([t for t in range(tA - 2)]
               + [t for t in range(tA, tB - 2)])
    j, wsum = 0, 0
    while wsum < SC:
        wsum += SC if nondiag[j] < tA else QB
        j += 1
    order = ([tA - 2, tA - 1] + nondiag[:j] + [tB - 2, tB - 1]
             + nondiag[j:])
    raw_groups, cur, off = [], [], 0
    for t in order:
        w = SC if t < tA else QB
        diag = t in (tA - 2, tA - 1, tB - 2, tB - 1)
        if off + w > SG * SC:
            raw_groups.append(cur)
            cur, off = [], 0
        cur.append((t, w, diag))
        off += w
    if cur:
        raw_groups.append(cur)
    t_first, t_last = order[0], order[-1]

    # Each group's tiles are split across TWO separate PSUM score tiles:
    # region A ([128,1024], exact exp on ACT) and region D ([128,512],
    # Schraudolph exp on DVE — unless a masked diagonal tile lands there,
    # in which case ACT handles D too). Separate tiles/banks per engine:
    # a single shared tile would serialize the ACT and DVE reads through
    # the PSUM bank-overlap tracker. Entries: (t, region, roff, w, diag).
    groups = []
    for cur in raw_groups:
        total = sum(w for _, w, _ in cur)
        dwidth, nd = 0, 0
        for t, w, diag in reversed(cur):
            if dwidth + w > SC:
                break
            dwidth += w
            nd += 1
        asz = total - dwidth
        assert asz <= 2 * SC and dwidth <= SC
        grp, aoff, doff = [], 0, 0
        for i, (t, w, diag) in enumerate(cur):
            if i < len(cur) - nd:
                grp.append((t, "A", aoff, w, diag))
                aoff += w
            else:
                grp.append((t, "D", doff, w, diag))
                doff += w
        groups.append((grp, aoff, doff,
                       any(dg for _, r, _, _, dg in grp if r == "D"),
                       any(dg for _, _, _, _, dg in grp)))

    with tile.TileContext(nc) as tc:
        with (
            tc.tile_pool(name="stat", bufs=1) as stat,
            tc.tile_pool(name="kp", bufs=2) as kp,
            tc.tile_pool(name="qp", bufs=3) as qp,
            tc.tile_pool(name="vp", bufs=2) as vp,
            tc.tile_pool(name="pp", bufs=8) as pp,
            tc.tile_pool(name="ppd", bufs=8) as ppd,
            tc.tile_pool(name="dp", bufs=4) as dp,
        ):
            # startup-critical loads first: statics, Q/K/V for head 0 only.
            st_sb = stat.tile([128, 2 * QB + 128], FP16, tag="st")
            nc.sync.dma_start(st_sb[:], St[:])
            mb0 = st_sb[:, 0:QB]
            mb1 = st_sb[:, QB:2 * QB]
            id_sb = st_sb[:, 2 * QB:]

            # DMA->tile deps are tile-granular (a reader waits for every DMA
            # into its tile), so Q gets one tile per head and K two per head.
            q_tiles, kv_tiles = {}, {}

            def load_q(h):
                qt_h = qp.tile([64, SC], FP16, tag="qt")
                nc.sync.dma_start(qt_h[:], Qt[:, h * SC:(h + 1) * SC])
                q_tiles[h] = qt_h

            load_q(0)

            # PE p-state warm-up during the K/V head-0 loads (see build_qkv)
            with tc.tile_pool(name="ps_w", bufs=1, space="PSUM") as ps_w:
                wu_ps = ps_w.tile([128, 128], F32, tag="wups")
                for _ in range(12):
                    nc.tensor.matmul(wu_ps[:], id_sb, id_sb,
                                     start=True, stop=True)

            def load_kv(h):
                kt_a = kp.tile([64, S // 2], FP16, tag="kta")
                nc.sync.dma_start(kt_a[:], Kt[h * 64:(h + 1) * 64, :S // 2])
                kt_b = kp.tile([64, S // 2], FP16, tag="ktb")
                nc.sync.dma_start(kt_b[:], Kt[h * 64:(h + 1) * 64, S // 2:])
                v_h = vp.tile([128, NKT * VW], FP16, tag="v")
                nc.sync.dma_start(
                    v_h[:], Vh[:, h * NKT * VW:(h + 1) * NKT * VW])
                kv_tiles[h] = (kt_a, kt_b, v_h)

            load_kv(0)

            # normalized attention output, natural layout:
            # [128 q, (qsub, h*64+d)] fp16
            attn_nat = stat.tile([128, 4 * D], FP16, tag="attn_nat")
            attn_bf = stat.tile([128, NT * SC], FP16, tag="attn")
            wot_sb = stat.tile([128, NT * D], FP16, tag="wot")

            def q_rhs(qt_h, qo, width):
                return qt_h[:, qo:qo + width]

            with (
                tc.tile_pool(name="ps_sa", bufs=2, space="PSUM") as ps_sa,
                tc.tile_pool(name="ps_sd", bufs=2, space="PSUM") as ps_sd,
                tc.tile_pool(name="ps_u", bufs=1, space="PSUM") as ps_u,
                tc.tile_pool(name="ps_t", bufs=1, space="PSUM") as ps_t,
            ):
                pending_fin = []
                for h in range(H):
                    kt_a, kt_b, v_h = kv_tiles.pop(h)
                    qt_h = q_tiles.pop(h)
                    # natural-layout AV accumulators, one per 128-q
                    # sub-tile, all four in ONE psum bank (4*65 = 260 f32).
                    # Only the very first mm uses start=True: it marks the
                    # whole 2KB bank pending-zero; the first write to each
                    # byte then overwrites, later writes accumulate.
                    unat = ps_u.tile([128, 512], F32, tag="u")

                    def av(t, p_slice, block, sub, _u=unat, _v=v_h):
                        uqo = (block * 2 + sub) * VW
                        nc.tensor.matmul(
                            _u[:, uqo:uqo + VW],
                            p_slice,
                            _v[:, t * VW:(t + 1) * VW],
                            start=(t == t_first and sub == 0 and block == 0),
                            stop=(t == t_last and block == 1 and sub == 1),
                            skip_group_check=True)

                    def emit_av(grp, p_act, p_dve):
                        for t, reg, roff, w, _ in grp:
                            p = p_act if reg == "A" else p_dve
                            for sub in (0, 1):
                                if w == SC:
                                    av(t, p[:, roff + sub * 128:
                                           roff + (sub + 1) * 128], 0, sub)
                                    av(t, p[:, roff + QB + sub * 128:
                                           roff + QB + (sub + 1) * 128],
                                       1, sub)
                                else:
                                    av(t, p[:, roff + sub * 128:
                                           roff + (sub + 1) * 128], 1, sub)

                    pends = []   # AV of group g is deferred four groups
                    for gi, (grp, wA, wD, d_diag, any_diag) in \
                            enumerate(groups):
                        sc_a = ps_sa.tile([128, 2 * SC], F32, tag="sa")
                        sc_d = ps_sd.tile([128, SC], F32, tag="sd")
                        for t, reg, roff, w, dg in grp:
                            sc = sc_a if reg == "A" else sc_d
                            nc.tensor.matmul(
                                sc[:, roff:roff + w],
                                (kt_a if t < NKT // 2 else kt_b)[
                                    :, (t % (NKT // 2)) * 128:
                                    (t % (NKT // 2) + 1) * 128],
                                q_rhs(qt_h, 0 if w == SC else QB, w),
                                start=True, stop=not dg)
                            if dg:
                                # additive causal mask (-60000 on the strict
                                # upper triangle), applied on the PE itself:
                                # identity-stationary matmul accumulates the
                                # mask tile into this score region
                                nc.tensor.matmul(
                                    sc[:, roff:roff + QB],
                                    id_sb,
                                    mb0 if t in (tA - 2, tB - 2) else mb1,
                                    start=False, stop=True,
                                    skip_group_check=True)
                        p_act = pp.tile([128, 2 * SC], FP16, tag="p")
                        p_dve = ppd.tile([128, SC], FP16, tag="pd")
                        # the (always diagonal-free) last group's A region
                        # also goes to DVE: evens out the ACT/DVE load
                        if gi == len(groups) - 1 and not any_diag:
                            nc.vector.tensor_scalar(
                                p_act[:, :wA].bitcast(I16),
                                sc_a[:, :wA], A_SCH, B_SCH,
                                op0=ALU.mult, op1=ALU.add)
                        else:
                            nc.scalar.activation(
                                p_act[:, :wA], sc_a[:, :wA], AF.Exp,
                                scale=0.125)
                        if wD:
                            if d_diag:
                                nc.scalar.activation(
                                    p_dve[:, :wD], sc_d[:, :wD], AF.Exp,
                                    scale=0.125)
                            else:
                                nc.vector.tensor_scalar(
                                    p_dve[:, :wD].bitcast(I16),
                                    sc_d[:, :wD], A_SCH, B_SCH,
                                    op0=ALU.mult, op1=ALU.add)
                        # prefetches + the previous head's deferred norm
                        # ride behind head h's first score group (emitting
                        # norm(h-1) here keeps it from head-of-line blocking
                        # head h's DVE exps in the in-order DVE queue; the
                        # straggler core does better one group later)
                        if gi == 0:
                            if h + 1 < H:
                                load_q(h + 1)
                                load_kv(h + 1)
                            if h == 0:
                                for g in range(NT):
                                    nc.sync.dma_start(
                                        wot_sb[:, g * D:(g + 1) * D],
                                        WoT[g * 128:(g + 1) * 128, :])
                        if gi == (1 if core == 2 else 0) and pending_fin:
                            pending_fin.pop()()

                        pends.append((grp, p_act, p_dve))
                        if len(pends) > 4:
                            emit_av(*pends.pop(0))
                    for pd in pends:
                        emit_av(*pd)

                    def finish_head(h=h, unat=unat):
                        # normalize: denominators are per-partition scalars
                        for block, sub in ((0, 0), (0, 1), (1, 0), (1, 1)):
                            qsub = block * 2 + sub
                            uqo = qsub * VW
                            r = dp.tile([128, 1], F32, tag="recip")
                            nc.vector.reciprocal(
                                r[:], unat[:, uqo + 64:uqo + 65])
                            nc.vector.tensor_scalar_mul(
                                attn_nat[:, qsub * D + h * DK:
                                         qsub * D + (h + 1) * DK],
                                unat[:, uqo:uqo + 64], r[:])
                        # transpose the finished head pair into W_o layout
                        if h % 2 == 1:
                            g = h // 2
                            for qsub in range(4):
                                tps = ps_t.tile([128, 128], FP16, tag="t")
                                nc.tensor.transpose(
                                    tps[:],
                                    attn_nat[:, qsub * D + g * 128:
                                             qsub * D + (g + 1) * 128],
                                    id_sb)
                                nc.vector.tensor_copy(
                                    attn_bf[:, g * SC + qsub * 128:
                                            g * SC + (qsub + 1) * 128],
                                    tps[:])

                    pending_fin.append(finish_head)
                if pending_fin:
                    pending_fin.pop()()

            # W_o: y^T[o-tile] = sum_c WoT[c-tile, o-tile]^T @ attn^T[c-tile].
            # ct-outer with all six o-accumulators live: the 30 matmuls over
            # head pairs 0-4 never sit in-order behind a chain that needs the
            # just-finished pair 5, so only the last 6 matmuls are exposed.
            with (
                tc.tile_pool(name="ps_y", bufs=4, space="PSUM") as ps_y,
                tc.tile_pool(name="yo", bufs=3) as yo,
            ):
                for o in range(NT):
                    yps = ps_y.tile([128, SC], F32, tag="y")
                    for ct in range(NT):
                        nc.tensor.matmul(
                            yps[:],
                            wot_sb[:, ct * D + o * 128:ct * D + (o + 1) * 128],
                            attn_bf[:, ct * SC:(ct + 1) * SC],
                            start=(ct == 0), stop=(ct == NT - 1))
                    yt_sb = yo.tile([128, SC], FP16, tag="yt")
                    nc.vector.tensor_copy(yt_sb[:], yps[:])
                    nc.sync.dma_start(yT[o * 128:(o + 1) * 128, :], yt_sb[:])
    nc.compile()
    return nc


# --------------------------------------------------------------------------
# Host-side packing + the public entry point
# --------------------------------------------------------------------------

def _make_statics():
    r = np.arange(128)[:, None]
    j = np.arange(QB)[None, :]
    st = np.empty((128, 2 * QB + 128), np.float16)
    st[:, 0:QB] = np.where(r > j, -60000.0, 0.0)          # additive mask m0
    st[:, QB:2 * QB] = np.where(128 + r > j, -60000.0, 0.0)  # m1
    st[:, 2 * QB:] = np.eye(128)
    return st


_programs = None


def _get_programs():
    global _programs
    if _programs is None:
        qkv = build_qkv()
        attn = [build_attn(c) for c in range(NC)]
        _programs = (qkv, attn)
    return _programs


def _pack_w_mk(W):
    """[768,768] torch Linear weight -> [128, 36*128] fp16, (m,k)-tile-major:
    out[p, (m*6+k)*128 + j] = W[m*128+j, k*128+p] (i.e. W^T by tiles)."""
    Wt = np.asarray(W, np.float32).T.astype(np.float16)       # [in k, out m]
    t = Wt.reshape(NT, 128, NT, 128)                           # [k, p, m, j]
    return np.ascontiguousarray(
        t.transpose(1, 2, 0, 3).reshape(128, NT * D))          # [p, m, k, j]


def kernel(x, W_q, W_k, W_v, W_o):
    x = np.asarray(x)
    in_dtype = x.dtype
    xs = np.asarray(x, np.float32).reshape(S, D)
    qkv_nc, attn_ncs = _get_programs()

    # ---- launch 1: QKV projections, sequence-sharded ----
    WqP, WkP = _pack_w_mk(W_q), _pack_w_mk(W_k)
    # WvP[p, k*768+n] = W_v[n, k*128+p]
    WvP = np.ascontiguousarray(
        np.asarray(W_v, np.float32).T.astype(np.float16)
        .reshape(NT, 128, D).transpose(1, 0, 2).reshape(128, NT * D))
    x16 = xs.astype(np.float16)
    in_maps1 = []
    for c in range(NC):
        xc = x16[c * SC:(c + 1) * SC]                          # [512, 768]
        xIc = np.ascontiguousarray(
            xc.T.reshape(NT, 128, SC).transpose(1, 0, 2).reshape(128, NT * SC))
        in_maps1.append({"xI": xIc, "WqP": WqP, "WkP": WkP, "WvP": WvP})
    res1 = run_mpmd([qkv_nc] * NC, in_maps1)

    # ---- host gather + repack ----
    # QtI/KtI images -> [768, 4096]; VnI image -> [4096, 768]
    Qt_full = np.empty((D, S), np.float16)
    Kt_full = np.empty((D, S), np.float16)
    V_full = np.empty((S, D), np.float16)
    for c in range(NC):
        q = res1[c]["QtI"].reshape(128, NT, SC).transpose(1, 0, 2)
        k = res1[c]["KtI"].reshape(128, NT, SC).transpose(1, 0, 2)
        Qt_full[:, c * SC:(c + 1) * SC] = q.reshape(D, SC)
        Kt_full[:, c * SC:(c + 1) * SC] = k.reshape(D, SC)
        v = res1[c]["VnI"].reshape(128, 4, D).transpose(1, 0, 2)
        V_full[c * SC:(c + 1) * SC] = v.reshape(SC, D)
    # per-head SBUF image of V augmented with a ones column:
    # Vh[p, h, t, e] = Vaug[t*128+p, h*65+e]
    Vaug = np.empty((NKT, 128, H, VW), np.float16)
    Vaug[:, :, :, :64] = V_full.reshape(NKT, 128, H, 64)
    Vaug[:, :, :, 64] = np.float16(1.0)
    Vh = np.ascontiguousarray(
        Vaug.transpose(1, 2, 0, 3).reshape(128, H * NKT * VW))
    st = _make_statics()

    # ---- launch 2: attention + W_o, query-sharded (zig-zag) ----
    WoT = np.ascontiguousarray(np.asarray(W_o, np.float32).T).astype(np.float16)
    in_maps2 = []
    for c in range(NC):
        bA, bB = _blocks_for_core(c)
        # per-head [64, 512] with that core's two query blocks side by side
        qh = np.empty((DK, H * SC), np.float16)
        for h in range(H):
            qh[:, h * SC:h * SC + QB] = \
                Qt_full[h * DK:(h + 1) * DK, bA * QB:(bA + 1) * QB]
            qh[:, h * SC + QB:(h + 1) * SC] = \
                Qt_full[h * DK:(h + 1) * DK, bB * QB:(bB + 1) * QB]
        in_maps2.append({
            "Qt": qh, "Kt": Kt_full, "Vh": Vh, "WoT": WoT, "St": st,
        })
    res2 = run_mpmd(attn_ncs, in_maps2)

    # ---- host scatter ----
    y = np.empty((S, D), np.float32)
    for c in range(NC):
        bA, bB = _blocks_for_core(c)
        yc = res2[c]["yT"].T.astype(np.float32)  # [512, 768]
        y[bA * QB:(bA + 1) * QB] = yc[:QB]
        y[bB * QB:(bB + 1) * QB] = yc[QB:]
    return y.reshape(B, S, D).astype(in_dtype, copy=False)
